# revision 1
# baseline (speedup 1.0000x reference)
"""MatchLSTM attention kernel for 8 Trainium2 NeuronCores.

Reference computation (B=64, T=2048, D=512):
    G   = tanh(input_p@Wp.T + bp + input_q@Wq.T + bq + h_tm1@Wr.T + br)
    a   = softmax(G@w + match_b)            over T
    z   = sum_t a[:,t] * input_q[:,:,t]
    out = concat([input_p, z], -1)

Sharding: data-parallel over batch, 8 batches per core, weights replicated.

Per-core device pipeline (all matmul operands bf16, fp32 accumulation):
  - c^T[o,b] = (Wp.T;Wr.T;bias) matmuls against (input_p^T;h^T;ones)  [once]
  - X^T tiles [q,tok] via DMA-transpose; X natural tiles [tok,q] via DMA
  - G^T[o,tok] = Wq.T-chunk @ X^T-chunk (PE, fp32 PSUM)
  - tanh via ScalarE with per-partition bias c^T  -> bf16 SBUF
  - scores s[1,tok] = w-chunk.T @ tanhG (PE accum over o-chunks)
  - s transposed to columns via K=1 fp16 matmuls; exp(s+match_b) on ScalarE
    -> bf16, with sumexp accumulated for free via activation accum_out
  - z[1,512] = sum_j esc_j.T @ Xnat_j (PE, fp32 PSUM accumulation)
  - z scaled by 1/sumexp (VectorE), DMA out.  Softmax max-subtraction is
    skipped: |s| <= sum|w| + 1 < 25, exp stays well inside fp32 range.
"""

import sys

if "/opt/trn_rl_repo" not in sys.path:
    sys.path.insert(0, "/opt/trn_rl_repo")

import numpy as np
import ml_dtypes

N_CORES = 8
B, T, D = 64, 2048, 512
PB = B // N_CORES          # batches per core
KC = D // 128              # 4 contraction chunks of 128
NTT = T // 512             # 4 token tiles of 512
NJ = T // 128              # 16 token chunks of 128
CROWS = 2 * D + 128        # cw/cx rows: Wp.T, Wr.T, bias row + zero pad

BF16 = ml_dtypes.bfloat16

_CACHE: dict = {}


def _build_program():
    import concourse.bacc as bacc
    import concourse.tile as tile
    import concourse.mybir as mybir
    from concourse.bass import MemorySpace

    dt = mybir.dt
    F32 = dt.float32
    BF = dt.bfloat16
    AF = mybir.ActivationFunctionType

    nc = bacc.Bacc(
        "TRN2", target_bir_lowering=False, debug=False, num_devices=N_CORES
    )

    xq_d = nc.dram_tensor("xq", [PB, T, D], BF, kind="ExternalInput")
    wq_d = nc.dram_tensor("wqt", [D, D], BF, kind="ExternalInput")      # Wq.T [q,o]
    cw_d = nc.dram_tensor("cw", [CROWS, D], BF, kind="ExternalInput")   # [Wp.T;Wr.T;bias;0]
    cx_d = nc.dram_tensor("cx", [CROWS, PB], BF, kind="ExternalInput")  # [ip.T;h.T;1;0]
    wcol_d = nc.dram_tensor("wcol", [D, 1], BF, kind="ExternalInput")
    mb_d = nc.dram_tensor("mb", [128, 1], F32, kind="ExternalInput")    # match_b bcast
    z_d = nc.dram_tensor("z", [1, PB * D], F32, kind="ExternalOutput")

    NKC = CROWS // 128  # 9 contraction chunks for the c matmuls

    F16 = dt.float16

    with tile.TileContext(nc) as tc:
        with (
            tc.tile_pool(name="consts", bufs=1) as consts,
            tc.tile_pool(name="xT_p", bufs=3) as xT_pool,
            tc.tile_pool(name="xnat_p", bufs=3) as xnat_pool,
            tc.tile_pool(name="tanh_p", bufs=8) as tanh_pool,
            tc.tile_pool(name="srow_p", bufs=3) as srow_pool,
            tc.tile_pool(name="esc_p", bufs=3) as esc_pool,
            tc.tile_pool(name="small_p", bufs=2) as small_pool,
            tc.tile_pool(name="zout_p", bufs=1) as zout_pool,
            tc.tile_pool(name="pG", bufs=2, space=MemorySpace.PSUM) as pG,
            tc.tile_pool(name="pS", bufs=2, space=MemorySpace.PSUM) as pS,
            tc.tile_pool(name="pZ", bufs=1, space=MemorySpace.PSUM) as pZ,
            tc.tile_pool(name="pM", bufs=1, space=MemorySpace.PSUM) as pM,
        ):
            # ---- constants (DMA order = criticality order) -----------------
            cw_s = consts.tile([128, NKC, D], BF, tag="cw", name="cw_s")
            nc.sync.dma_start(out=cw_s, in_=cw_d.rearrange("(c p) o -> p c o", p=128))
            cx_s = consts.tile([128, NKC, PB], BF, tag="cx", name="cx_s")
            nc.sync.dma_start(out=cx_s, in_=cx_d.rearrange("(c p) b -> p c b", p=128))
            wq_s = consts.tile([128, KC, D], BF, tag="wq", name="wq_s")
            nc.sync.dma_start(out=wq_s, in_=wq_d.rearrange("(c p) o -> p c o", p=128))
            wcol_s = consts.tile([128, KC, 1], BF, tag="wcol", name="wcol_s")
            nc.sync.dma_start(out=wcol_s, in_=wcol_d.rearrange("(c p) o -> p c o", p=128))
            mb_s = consts.tile([128, 1], F32, tag="mb", name="mb_s")
            nc.sync.dma_start(out=mb_s, in_=mb_d[:, :])
            ones128 = consts.tile([128, 1], F32, tag="ones128", name="ones128")
            nc.vector.memset(ones128, 1.0)
            ones_f16 = consts.tile([1, 1], F16, tag="ones_f16", name="ones_f16")
            nc.vector.memset(ones_f16, 1.0)
            # warm the ACT table set (tanh/exp share one set) off the critical path
            dummy_s = consts.tile([1, 1], F32, tag="dummy", name="dummy_s")
            nc.scalar.activation(
                out=dummy_s, in_=ones_f16, func=AF.Tanh, bias=0.0, scale=1.0
            )

            # ---- c^T[o, b] for all batches (once) --------------------------
            c_ps = pM.tile([128, KC, PB], F32, tag="misc", name="c_ps")
            for oc in range(KC):
                for k in range(NKC):
                    nc.tensor.matmul(
                        c_ps[:, oc, :],
                        cw_s[:, k, oc * 128 : (oc + 1) * 128],
                        cx_s[:, k, :],
                        start=(k == 0),
                        stop=(k == NKC - 1),
                    )
            cT_s = consts.tile([128, KC, PB], F32, tag="cT", name="cT_s")
            nc.vector.tensor_copy(out=cT_s, in_=c_ps)

            zout_s = zout_pool.tile([1, PB, D], F32, tag="zout", name="zout_s")

            # ---- per-batch pipeline ---------------------------------------
            for b in range(PB):
                xT = xT_pool.tile([128, KC, T], BF, tag="xT", name="xT")
                # batch 0 is latency-critical: land the first half-T of each
                # q-chunk sooner by splitting the transposes.
                nh = 2 if b == 0 else 1
                for h in range(nh):
                    for qc in range(KC):
                        nc.sync.dma_start(
                            out=xT[:, qc, h * (T // nh) : (h + 1) * (T // nh)],
                            in_=xq_d[
                                b,
                                h * (T // nh) : (h + 1) * (T // nh),
                                qc * 128 : (qc + 1) * 128,
                            ],
                            transpose=True,
                        )
                xnat = xnat_pool.tile([128, NJ, D], BF, tag="xnat", name="xnat")
                nc.sync.dma_start(
                    out=xnat, in_=xq_d[b].rearrange("(i p) q -> p i q", p=128)
                )

                s_cat = srow_pool.tile([1, T], F16, tag="scat", name="s_cat")
                esc = esc_pool.tile([128, NJ], BF, tag="esc", name="esc")
                pesum = small_pool.tile([128, 2], F32, tag="pesum", name="pesum")
                z_ps = pZ.tile([1, D], F32, tag="z", name="z_ps")
                # token tiles processed in pairs sharing one [128,1024] PSUM
                # G tile (2 banks): same Wq chunk stays loaded across the pair
                # and tanh runs once per 1024 tokens.
                for tp in range(NTT // 2):
                    sT_ps = pM.tile([128, NJ // 2], F32, tag="misc", name="sT_ps")
                    sc_pair = [
                        pS.tile([1, 512], F32, tag="s", name="sc_ps")
                        for _ in range(2)
                    ]
                    for oc in range(KC):
                        g_ps = pG.tile([128, 1024], F32, tag="g", name="g_ps")
                        for qc in range(KC):
                            for i in range(2):
                                tt = tp * 2 + i
                                nc.tensor.matmul(
                                    g_ps[:, i * 512 : (i + 1) * 512],
                                    wq_s[:, qc, oc * 128 : (oc + 1) * 128],
                                    xT[:, qc, tt * 512 : (tt + 1) * 512],
                                    start=(qc == 0),
                                    stop=(qc == KC - 1),
                                )
                        th = tanh_pool.tile([128, 1024], BF, tag="th", name="th")
                        nc.scalar.activation(
                            out=th,
                            in_=g_ps,
                            func=AF.Tanh,
                            bias=cT_s[:, oc, b : b + 1],
                            scale=1.0,
                        )
                        for i in range(2):
                            nc.tensor.matmul(
                                sc_pair[i],
                                wcol_s[:, oc, :],
                                th[:, i * 512 : (i + 1) * 512],
                                start=(oc == 0),
                                stop=(oc == KC - 1),
                            )
                    for i in range(2):
                        tt = tp * 2 + i
                        nc.vector.tensor_copy(
                            out=s_cat[:, tt * 512 : (tt + 1) * 512], in_=sc_pair[i]
                        )
                        # transpose scores into columns (K=1 fp16 matmuls)
                        for jj in range(4):
                            j = tt * 4 + jj
                            nc.tensor.matmul(
                                sT_ps[:, j - tp * 8 : j - tp * 8 + 1],
                                s_cat[:, j * 128 : (j + 1) * 128],
                                ones_f16,
                                start=True,
                                stop=True,
                            )
                    # exp + its half of the z accumulation start mid-batch
                    nc.scalar.activation(
                        out=esc[:, tp * 8 : (tp + 1) * 8],
                        in_=sT_ps,
                        func=AF.Exp,
                        bias=mb_s,
                        scale=1.0,
                        accum_out=pesum[:, tp : tp + 1],
                    )
                    for j in range(tp * 8, (tp + 1) * 8):
                        nc.tensor.matmul(
                            z_ps,
                            esc[:, j : j + 1],
                            xnat[:, j, :],
                            start=(j == 0),
                            stop=(j == NJ - 1),
                        )

                se_ps = pM.tile([1, 2], F32, tag="misc", name="se_ps")
                nc.tensor.matmul(se_ps, ones128, pesum, start=True, stop=True)
                se_sb = small_pool.tile([1, 2], F32, tag="sesb", name="se_sb")
                nc.vector.tensor_copy(out=se_sb, in_=se_ps)
                se_tot = small_pool.tile([1, 1], F32, tag="setot", name="se_tot")
                nc.vector.tensor_add(se_tot, se_sb[:, 0:1], se_sb[:, 1:2])
                rse_s = small_pool.tile([1, 1], F32, tag="rse", name="rse_s")
                nc.vector.reciprocal(out=rse_s, in_=se_tot)

                nc.vector.tensor_scalar_mul(
                    out=zout_s[:, b, :], in0=z_ps, scalar1=rse_s
                )

            nc.sync.dma_start(out=z_d[:, :], in_=zout_s)

    nc.compile()
    return nc


def _get_program():
    if "nc" not in _CACHE:
        _CACHE["nc"] = _build_program()
    return _CACHE["nc"]


def kernel(**inputs) -> np.ndarray:
    from concourse import bass_utils

    inp = {k: np.asarray(v) for k, v in inputs.items()}
    input_p = inp["input_p"].astype(np.float32)
    input_q = inp["input_q"].astype(np.float32)
    h_tm1 = inp["h_tm1"].astype(np.float32)
    Wp, Wq, Wr = inp["Wp"], inp["Wq"], inp["Wr"]
    bp, bq, br = inp["bp"], inp["bq"], inp["br"]
    w = inp["w"]
    mb = float(np.asarray(inp["match_b"]).reshape(-1)[0])

    # shared (weight) tensors
    wqt = np.ascontiguousarray(Wq.T).astype(BF16)
    cw = np.zeros((CROWS, D), dtype=BF16)
    cw[:D] = Wp.T.astype(BF16)
    cw[D : 2 * D] = Wr.T.astype(BF16)
    cw[2 * D] = (bp.astype(np.float32) + bq + br).astype(BF16)
    wcol = np.ascontiguousarray(w.reshape(D, 1)).astype(BF16)
    mb_arr = np.full((128, 1), mb, dtype=np.float32)

    nc = _get_program()

    in_maps = []
    for c in range(N_CORES):
        s = slice(c * PB, (c + 1) * PB)
        cx = np.zeros((CROWS, PB), dtype=BF16)
        cx[:D] = input_p[s].T.astype(BF16)
        cx[D : 2 * D] = h_tm1[s].T.astype(BF16)
        cx[2 * D] = 1.0
        in_maps.append(
            {
                "xq": np.ascontiguousarray(input_q[s]).astype(BF16),
                "wqt": wqt,
                "cw": cw,
                "cx": cx,
                "wcol": wcol,
                "mb": mb_arr,
            }
        )

    res = bass_utils.run_bass_kernel_spmd(
        nc, in_maps, core_ids=list(range(N_CORES))
    )
    z = np.concatenate(
        [
            np.asarray(res.results[c]["z"], dtype=np.float32).reshape(PB, D)
            for c in range(N_CORES)
        ],
        axis=0,
    )
    return np.concatenate([input_p, z], axis=1)



# revision 2
# speedup vs baseline: 2.3058x; 2.3058x over previous
"""MatchLSTM attention kernel for 8 Trainium2 NeuronCores.

Reference computation (B=64, T=2048, D=512):
    G   = tanh(input_p@Wp.T + bp + input_q@Wq.T + bq + h_tm1@Wr.T + br)
    a   = softmax(G@w + match_b)            over T
    z   = sum_t a[:,t] * input_q[:,t,:]
    out = concat([input_p, z], -1)

Sharding: data-parallel over batch, 8 batches per core, weights replicated.

Per-core device pipeline:
  - c^T[o,b] = (Wp.T;Wr.T;bias) matmuls against (ip^T;h^T;ones)  [once]
  - G^T[o,tok] tiles via fp8e4 DoubleRow matmuls (K=256 per instr, 0.5
    cyc/row): Wq^T-chunks (stationary) x X^T-chunks (moving), fp32 PSUM
  - tanh on ScalarE with per-partition bias c^T -> bf16 SBUF  [ACT is the
    bottleneck engine: 8192 free-elems/batch at 0.833 ns/elem]
  - scores come out TRANSPOSED for free: lhsT = tanh-tile [o,128 toks],
    rhs = w-chunk [o,1] -> sT[128,1] per token chunk (1-row matmuls,
    PSUM-accumulated over the 4 o-chunks)
  - one exp per batch on sT[128,16] -> fp8 esc, sumexp via accum_out
  - z^T[q,1] per q-chunk: lhsT = X-natural chunk [tok,128 q], rhs =
    esc[:,j] (1-row matmuls accumulated over the 16 token chunks)
  - sum(pesum) via ones-matmul, reciprocal + scale on VectorE, DMA out.
    Softmax max-subtraction skipped: |s| <= sum|w|+1 < 25, exp safe.
  - X is staged twice from host (fp8 transposed for G, fp8 natural for z)
    so no DMA-transpose is needed; scores/exp/z of batch b-1 are emitted
    inside batch b's G phase so ACT never idles.
"""

import sys

if "/opt/trn_rl_repo" not in sys.path:
    sys.path.insert(0, "/opt/trn_rl_repo")

import numpy as np
import ml_dtypes

N_CORES = 8
B, T, D = 64, 2048, 512
PB = B // N_CORES          # batches per core
KC = D // 128              # 4 contraction / o / q chunks of 128
NJ = T // 128              # 16 token chunks of 128
CROWS = 2 * D + 128        # cw/cx rows: Wp.T, Wr.T, bias row + zero pad
NKC = CROWS // 128         # 9 contraction chunks for the c matmuls

BF16 = ml_dtypes.bfloat16
FP8 = ml_dtypes.float8_e4m3

_CACHE: dict = {}


def _build_program():
    import concourse.bacc as bacc
    import concourse.tile as tile
    import concourse.mybir as mybir
    from concourse.bass import MemorySpace

    dt = mybir.dt
    F32 = dt.float32
    BF = dt.bfloat16
    F8 = dt.float8e4
    AF = mybir.ActivationFunctionType
    DR = mybir.MatmulPerfMode.DoubleRow

    nc = bacc.Bacc(
        "TRN2", target_bir_lowering=False, debug=False, num_devices=N_CORES
    )

    xt_d = nc.dram_tensor("xt8", [PB, D, T], F8, kind="ExternalInput")     # X^T
    xn_d = nc.dram_tensor("xn8", [PB, T, D], F8, kind="ExternalInput")     # X
    wq_d = nc.dram_tensor("wq8", [D, D], F8, kind="ExternalInput")         # Wq.T [q,o]
    cw_d = nc.dram_tensor("cw", [CROWS, D], BF, kind="ExternalInput")      # [Wp.T;Wr.T;bias;0]
    cx_d = nc.dram_tensor("cx", [CROWS, PB], BF, kind="ExternalInput")     # [ip.T;h.T;1;0]
    wcol_d = nc.dram_tensor("wcol", [D, 1], BF, kind="ExternalInput")
    z_d = nc.dram_tensor("z", [128, PB * KC], F32, kind="ExternalOutput")  # z^T chunks

    with tile.TileContext(nc) as tc:
        with (
            tc.tile_pool(name="consts", bufs=1) as consts,
            tc.tile_pool(name="xT_p", bufs=3) as xT_pool,
            tc.tile_pool(name="xn_p", bufs=3) as xn_pool,
            tc.tile_pool(name="th_p", bufs=10) as th_pool,
            tc.tile_pool(name="esc_p", bufs=2) as esc_pool,
            tc.tile_pool(name="small_p", bufs=2) as small_pool,
            tc.tile_pool(name="zout_p", bufs=1) as zout_pool,
            tc.tile_pool(name="pG", bufs=3, space=MemorySpace.PSUM) as pG,
            tc.tile_pool(name="pSZ", bufs=2, space=MemorySpace.PSUM) as pSZ,
        ):
            # ---- constants (DMA order = criticality order) -----------------
            cw_s = consts.tile([128, NKC, D], BF, tag="cw", name="cw_s")
            nc.sync.dma_start(out=cw_s, in_=cw_d.rearrange("(c p) o -> p c o", p=128))
            cx_s = consts.tile([128, NKC, PB], BF, tag="cx", name="cx_s")
            nc.sync.dma_start(out=cx_s, in_=cx_d.rearrange("(c p) b -> p c b", p=128))
            wq_s = consts.tile([128, KC, D], F8, tag="wq", name="wq_s")
            nc.sync.dma_start(out=wq_s, in_=wq_d.rearrange("(c p) o -> p c o", p=128))
            wcol_s = consts.tile([128, KC, 1], BF, tag="wcol", name="wcol_s")
            nc.sync.dma_start(out=wcol_s, in_=wcol_d.rearrange("(c p) o -> p c o", p=128))
            ones_f32 = consts.tile([128, 128], F32, tag="ones", name="ones_f32")
            nc.vector.memset(ones_f32, 1.0)
            # warm the ACT table set (tanh/exp share one set)
            dummy_s = consts.tile([1, 1], F32, tag="dummy", name="dummy_s")
            nc.scalar.activation(
                out=dummy_s, in_=ones_f32[0:1, 0:1], func=AF.Tanh, bias=0.0, scale=1.0
            )

            # ---- c^T[o, b] for all batches (once) --------------------------
            c_ps = pSZ.tile([128, KC, PB], F32, tag="sz", name="c_ps")
            for oc in range(KC):
                for k in range(NKC):
                    nc.tensor.matmul(
                        c_ps[:, oc, :],
                        cw_s[:, k, oc * 128 : (oc + 1) * 128],
                        cx_s[:, k, :],
                        start=(k == 0),
                        stop=(k == NKC - 1),
                    )
            cT_s = consts.tile([128, KC, PB], F32, tag="cT", name="cT_s")
            nc.vector.tensor_copy(out=cT_s, in_=c_ps)

            zout_s = zout_pool.tile([128, PB, KC], F32, tag="zout", name="zout_s")

            # per-batch state carried across the software pipeline
            xT = [None] * PB      # [128, KC, T] fp8 transposed X
            xn = [None] * PB      # [128, NJ, D] fp8 natural X
            th = [[None] * (2 * KC) for _ in range(PB)]  # tanh tiles (h*KC+oc)
            sz = [None] * PB      # [128, 21] psum: sT 0:16, zT 16:20, se 20
            esc = [None] * PB     # [128, NJ] fp8 exp scores
            pesum = [None] * PB   # [128, 1] f32 partial sumexp

            def dma_in(b, split):
                xT[b] = xT_pool.tile([128, KC, T], F8, tag="xT", name="xT")
                nsp = 4 if split else 1
                for s in range(nsp):
                    t0, t1 = s * (T // nsp), (s + 1) * (T // nsp)
                    nc.sync.dma_start(
                        out=xT[b][:, :, t0:t1],
                        in_=xt_d[b, :, t0:t1].rearrange("(c p) t -> p c t", p=128),
                    )
                xn[b] = xn_pool.tile([128, NJ, D], F8, tag="xn", name="xn")
                nc.sync.dma_start(
                    out=xn[b], in_=xn_d[b].rearrange("(j p) q -> p j q", p=128)
                )

            def emit_g_tanh(b, h, oc):
                g = pG.tile([128, 1024], F32, tag="g", name="g_ps")
                for kk in range(KC // 2):
                    for i in range(2):
                        t0 = h * 1024 + i * 512
                        nc.tensor.matmul(
                            g[:, i * 512 : (i + 1) * 512],
                            wq_s[:, 2 * kk : 2 * kk + 2, oc * 128 : (oc + 1) * 128],
                            xT[b][:, 2 * kk : 2 * kk + 2, t0 : t0 + 512],
                            start=(kk == 0),
                            stop=(kk == KC // 2 - 1),
                            perf_mode=DR,
                        )
                t = th_pool.tile([128, 1024], BF, tag="th", name="th")
                nc.scalar.activation(
                    out=t, in_=g, func=AF.Tanh,
                    bias=cT_s[:, oc, b : b + 1], scale=1.0,
                )
                th[b][h * KC + oc] = t

            def emit_scores(b, h):
                if sz[b] is None:
                    sz[b] = pSZ.tile([128, 21], F32, tag="sz", name="sz_ps")
                for jj in range(8):
                    j = h * 8 + jj
                    for oc in range(KC):
                        nc.tensor.matmul(
                            sz[b][:, j : j + 1],
                            th[b][h * KC + oc][:, jj * 128 : (jj + 1) * 128],
                            wcol_s[:, oc, :],
                            start=(oc == 0),
                            stop=(oc == KC - 1),
                            skip_group_check=True,
                        )

            def emit_exp(b):
                esc[b] = esc_pool.tile([128, NJ], F8, tag="esc", name="esc")
                pesum[b] = small_pool.tile([128, 1], F32, tag="pesum", name="pesum")
                nc.scalar.activation(
                    out=esc[b], in_=sz[b][:, 0:NJ], func=AF.Exp,
                    bias=0.0, scale=1.0, accum_out=pesum[b],
                )

            def emit_z_tail(b):
                for j in range(NJ):
                    for qc in range(KC):
                        nc.tensor.matmul(
                            sz[b][:, NJ + qc : NJ + qc + 1],
                            xn[b][:, j, qc * 128 : (qc + 1) * 128],
                            esc[b][:, j : j + 1],
                            start=(j == 0),
                            stop=(j == NJ - 1),
                            skip_group_check=True,
                        )
                nc.tensor.matmul(
                    sz[b][:, 20:21], ones_f32, pesum[b], start=True, stop=True,
                    skip_group_check=True,
                )
                rse = small_pool.tile([128, 1], F32, tag="rse", name="rse")
                nc.vector.reciprocal(out=rse, in_=sz[b][:, 20:21])
                nc.vector.tensor_scalar_mul(
                    out=zout_s[:, b, :], in0=sz[b][:, NJ : NJ + KC], scalar1=rse
                )

            # ---- software-pipelined batch loop ----------------------------
            dma_in(0, split=True)
            dma_in(1, split=False)
            for b in range(PB):
                if b + 2 < PB:
                    dma_in(b + 2, split=False)
                # half 0
                for oc in range(KC):
                    emit_g_tanh(b, 0, oc)
                    if oc == 1 and b > 0:
                        emit_scores(b - 1, 1)
                        emit_exp(b - 1)
                    if oc == 3 and b > 0:
                        emit_z_tail(b - 1)
                # half 1
                for oc in range(KC):
                    emit_g_tanh(b, 1, oc)
                    if oc == 1:
                        emit_scores(b, 0)
            # drain the pipeline for the last batch
            emit_scores(PB - 1, 1)
            emit_exp(PB - 1)
            emit_z_tail(PB - 1)

            nc.sync.dma_start(out=z_d[:, :], in_=zout_s)

    nc.compile()
    return nc


def _get_program():
    if "nc" not in _CACHE:
        _CACHE["nc"] = _build_program()
    return _CACHE["nc"]


def kernel(**inputs) -> np.ndarray:
    from concourse import bass_utils

    inp = {k: np.asarray(v) for k, v in inputs.items()}
    input_p = inp["input_p"].astype(np.float32)
    input_q = inp["input_q"].astype(np.float32)
    h_tm1 = inp["h_tm1"].astype(np.float32)
    Wp, Wq, Wr = inp["Wp"], inp["Wq"], inp["Wr"]
    bp, bq, br = inp["bp"], inp["bq"], inp["br"]
    w = inp["w"]

    # shared (weight) tensors
    wq8 = np.ascontiguousarray(Wq.T).astype(FP8)
    cw = np.zeros((CROWS, D), dtype=BF16)
    cw[:D] = Wp.T.astype(BF16)
    cw[D : 2 * D] = Wr.T.astype(BF16)
    cw[2 * D] = (bp.astype(np.float32) + bq + br).astype(BF16)
    wcol = np.ascontiguousarray(w.reshape(D, 1)).astype(BF16)

    nc = _get_program()

    in_maps = []
    for c in range(N_CORES):
        s = slice(c * PB, (c + 1) * PB)
        cx = np.zeros((CROWS, PB), dtype=BF16)
        cx[:D] = input_p[s].T.astype(BF16)
        cx[D : 2 * D] = h_tm1[s].T.astype(BF16)
        cx[2 * D] = 1.0
        xn8 = input_q[s].astype(FP8)
        xt8 = np.ascontiguousarray(xn8.transpose(0, 2, 1))
        in_maps.append(
            {
                "xt8": xt8,
                "xn8": xn8,
                "wq8": wq8,
                "cw": cw,
                "cx": cx,
                "wcol": wcol,
            }
        )

    res = bass_utils.run_bass_kernel_spmd(
        nc, in_maps, core_ids=list(range(N_CORES))
    )
    zs = []
    for c in range(N_CORES):
        zt = np.asarray(res.results[c]["z"], dtype=np.float32).reshape(128, PB, KC)
        zs.append(zt.transpose(1, 2, 0).reshape(PB, D))
    z = np.concatenate(zs, axis=0)
    return np.concatenate([input_p, z], axis=1)


# revision 17
# speedup vs baseline: 2.3739x; 1.0295x over previous
"""MatchLSTM attention kernel for 8 Trainium2 NeuronCores.

Reference computation (B=64, T=2048, D=512):
    G   = tanh(input_p@Wp.T + bp + input_q@Wq.T + bq + h_tm1@Wr.T + br)
    a   = softmax(G@w + match_b)            over T
    z   = sum_t a[:,t] * input_q[:,t,:]
    out = concat([input_p, z], -1)

Sharding: data-parallel over batch, 8 batches per core, weights replicated.

Per-core device pipeline:
  - c^T[o,b] = (Wp.T;Wr.T;bias) matmuls against (ip^T;h^T;ones)  [once]
  - G^T[o,tok] tiles via fp8e4 DoubleRow matmuls (K=256 per instr, 0.5
    cyc/row): Wq^T-chunks (stationary) x X^T-chunks (moving), fp32 PSUM
  - tanh on ScalarE with per-partition bias c^T -> bf16 SBUF  [ACT is the
    bottleneck engine: 8192 free-elems/batch at 0.833 ns/elem]
  - scores come out TRANSPOSED for free: lhsT = tanh-tile [o,128 toks],
    rhs = w-chunk [o,1] -> sT[128,1] per token chunk (1-row matmuls,
    PSUM-accumulated over the 4 o-chunks)
  - one exp per batch on sT[128,16] -> fp8 esc, sumexp via accum_out
  - z^T[q,1] per q-chunk: lhsT = X-natural chunk [tok,128 q], rhs =
    esc[:,j] (1-row matmuls accumulated over the 16 token chunks)
  - sum(pesum) via ones-matmul, reciprocal + scale on VectorE, DMA out.
    Softmax max-subtraction skipped: |s| <= sum|w|+1 < 25, exp safe.
  - X is staged twice from host (fp8 transposed for G, fp8 natural for z)
    so no DMA-transpose is needed; scores/exp/z of batch b-1 are emitted
    inside batch b's G phase so ACT never idles.
"""

import sys

if "/opt/trn_rl_repo" not in sys.path:
    sys.path.insert(0, "/opt/trn_rl_repo")

import numpy as np
import ml_dtypes

N_CORES = 8
B, T, D = 64, 2048, 512
PB = B // N_CORES          # batches per core
KC = D // 128              # 4 contraction / o / q chunks of 128
NJ = T // 128              # 16 token chunks of 128
CROWS = 2 * D + 128        # cw/cx rows: Wp.T, Wr.T, bias row + zero pad
NKC = CROWS // 128         # 9 contraction chunks for the c matmuls

BF16 = ml_dtypes.bfloat16
FP8 = ml_dtypes.float8_e4m3

CW_FP8 = True  # c-projection weights in fp8 (faster startup DMA)

_CACHE: dict = {}


def _build_program():
    import concourse.bacc as bacc
    import concourse.tile as tile
    import concourse.mybir as mybir
    from concourse.bass import MemorySpace

    dt = mybir.dt
    F32 = dt.float32
    BF = dt.bfloat16
    F8 = dt.float8e4
    AF = mybir.ActivationFunctionType
    DR = mybir.MatmulPerfMode.DoubleRow

    nc = bacc.Bacc(
        "TRN2", target_bir_lowering=False, debug=False, num_devices=N_CORES
    )

    xt_d = nc.dram_tensor("xt8", [PB, D, T], F8, kind="ExternalInput")     # X^T
    xn_d = nc.dram_tensor("xn8", [PB, T, D], F8, kind="ExternalInput")     # X
    wq_d = nc.dram_tensor("wq8", [D, D], F8, kind="ExternalInput")         # Wq.T [q,o]
    CWD = F8 if CW_FP8 else BF
    cw_d = nc.dram_tensor("cw", [CROWS, D], CWD, kind="ExternalInput")     # [Wp.T;Wr.T;bias;0]
    cx_d = nc.dram_tensor("cx", [CROWS, PB], CWD, kind="ExternalInput")    # [ip.T;h.T;1;0]
    wcol_d = nc.dram_tensor("wcol", [D, 1], BF, kind="ExternalInput")
    z_d = nc.dram_tensor("z", [128, PB * KC], F32, kind="ExternalOutput")  # z^T chunks

    with tile.TileContext(nc) as tc:
        with (
            tc.tile_pool(name="consts", bufs=1) as consts,
            tc.tile_pool(name="xT_p", bufs=3) as xT_pool,
            tc.tile_pool(name="xn_p", bufs=3) as xn_pool,
            tc.tile_pool(name="th_p", bufs=10) as th_pool,
            tc.tile_pool(name="esc_p", bufs=2) as esc_pool,
            tc.tile_pool(name="small_p", bufs=2) as small_pool,
            tc.tile_pool(name="zout_p", bufs=1) as zout_pool,
            tc.tile_pool(name="pG", bufs=3, space=MemorySpace.PSUM) as pG,
            tc.tile_pool(name="pSZ", bufs=2, space=MemorySpace.PSUM) as pSZ,
        ):
            # ---- constants + batch-0 DMAs, interleaved so the critical path
            # to the first tanh (cw+cx for the bias, then wq8+xT0[t0:1024]
            # for G) clears the serialized DMA device as early as possible --
            cw_s = consts.tile([128, NKC, D], F8 if CW_FP8 else BF, tag="cw", name="cw_s")
            nc.sync.dma_start(out=cw_s, in_=cw_d.rearrange("(c p) o -> p c o", p=128))
            cx_s = consts.tile([128, NKC, PB], F8 if CW_FP8 else BF, tag="cx", name="cx_s")
            nc.sync.dma_start(out=cx_s, in_=cx_d.rearrange("(c p) b -> p c b", p=128))
            wq_s = consts.tile([128, KC, D], F8, tag="wq", name="wq_s")
            nc.sync.dma_start(out=wq_s, in_=wq_d.rearrange("(c p) o -> p c o", p=128))
            xT0 = xT_pool.tile([128, KC, T], F8, tag="xT", name="xT")
            nc.sync.dma_start(
                out=xT0[:, :, 0:1024],
                in_=xt_d[0, :, 0:1024].rearrange("(c p) t -> p c t", p=128),
            )
            wcol_s = consts.tile([128, KC, 1], BF, tag="wcol", name="wcol_s")
            nc.sync.dma_start(out=wcol_s, in_=wcol_d.rearrange("(c p) o -> p c o", p=128))
            nc.sync.dma_start(
                out=xT0[:, :, 1024:2048],
                in_=xt_d[0, :, 1024:2048].rearrange("(c p) t -> p c t", p=128),
            )
            ones_bf = consts.tile([128, 128], BF, tag="ones", name="ones_bf")
            nc.vector.memset(ones_bf, 1.0)
            # warm the ACT table set (tanh/exp share one set)
            dummy_s = consts.tile([1, 1], F32, tag="dummy", name="dummy_s")
            nc.scalar.activation(
                out=dummy_s, in_=ones_bf[0:1, 0:1], func=AF.Tanh, bias=0.0, scale=1.0
            )

            # ---- c^T[o, b] for all batches (once) --------------------------
            c_ps = pSZ.tile([128, KC, PB], F32, tag="sz", name="c_ps")
            for oc in range(KC):
                for k in range(NKC):
                    nc.tensor.matmul(
                        c_ps[:, oc, :],
                        cw_s[:, k, oc * 128 : (oc + 1) * 128],
                        cx_s[:, k, :],
                        start=(k == 0),
                        stop=(k == NKC - 1),
                    )
            cT_s = consts.tile([128, KC, PB], F32, tag="cT", name="cT_s")
            nc.vector.tensor_copy(out=cT_s, in_=c_ps)

            zout_s = zout_pool.tile([128, PB, KC], F32, tag="zout", name="zout_s")

            # per-batch state carried across the software pipeline
            xT = [None] * PB      # [128, KC, T] fp8 transposed X
            xn = [None] * PB      # [128, NJ, D] fp8 natural X
            th = [[None] * (2 * KC) for _ in range(PB)]  # tanh tiles (h*KC+oc)
            sz = [None] * PB      # [128,37] psum: sT 0:16, colsum 16:32, zT 32:36
            esc = [None] * PB     # [128, NJ] exp scores
            sesb = [None] * PB    # [128, 1] f32 sumexp (all partitions)
            xT[0] = xT0

            def dma_in(b):
                xT[b] = xT_pool.tile([128, KC, T], F8, tag="xT", name="xT")
                nc.sync.dma_start(
                    out=xT[b], in_=xt_d[b].rearrange("(c p) t -> p c t", p=128)
                )
                xn[b] = xn_pool.tile([128, NJ, D], F8, tag="xn", name="xn")
                nc.sync.dma_start(
                    out=xn[b], in_=xn_d[b].rearrange("(j p) q -> p j q", p=128)
                )

            def emit_g_tanh(b, h, oc):
                g = pG.tile([128, 1024], F32, tag="g", name="g_ps")
                for kk in range(KC // 2):
                    for i in range(2):
                        t0 = h * 1024 + i * 512
                        nc.tensor.matmul(
                            g[:, i * 512 : (i + 1) * 512],
                            wq_s[:, 2 * kk : 2 * kk + 2, oc * 128 : (oc + 1) * 128],
                            xT[b][:, 2 * kk : 2 * kk + 2, t0 : t0 + 512],
                            start=(kk == 0),
                            stop=(kk == KC // 2 - 1),
                            perf_mode=DR,
                        )
                t = th_pool.tile([128, 1024], BF, tag="th", name="th")
                nc.scalar.activation(
                    out=t, in_=g, func=AF.Tanh,
                    bias=cT_s[:, oc, b : b + 1], scale=1.0,
                )
                th[b][h * KC + oc] = t

            def emit_scores(b, h):
                if sz[b] is None:
                    sz[b] = pSZ.tile([128, 37], F32, tag="sz", name="sz_ps")
                for jj in range(8):
                    j = h * 8 + jj
                    for oc in range(KC):
                        nc.tensor.matmul(
                            sz[b][:, j : j + 1],
                            th[b][h * KC + oc][:, jj * 128 : (jj + 1) * 128],
                            wcol_s[:, oc, :],
                            start=(oc == 0),
                            stop=(oc == KC - 1),
                            skip_group_check=True,
                        )

            def emit_exp(b):
                esc[b] = esc_pool.tile([128, NJ], BF, tag="esc", name="esc")
                nc.scalar.activation(
                    out=esc[b], in_=sz[b][:, 0:NJ], func=AF.Exp,
                    bias=0.0, scale=1.0,
                )
                # sumexp: per-chunk partition sums on PE (every out partition
                # gets all 16 chunk sums), then a free-dim reduce on DVE
                nc.tensor.matmul(
                    sz[b][:, 16:32], ones_bf, esc[b], start=True, stop=True,
                    skip_group_check=True,
                )
                sesb[b] = small_pool.tile([128, 1], F32, tag="sesb", name="sesb")
                nc.vector.tensor_reduce(
                    out=sesb[b], in_=sz[b][:, 16:32],
                    axis=mybir.AxisListType.X, op=mybir.AluOpType.add,
                )

            def emit_z_tail(b):
                # qc outer: each PSUM column's accumulation group must run
                # start->stop without another group opening in the same bank
                for qc in range(KC):
                    for j in range(NJ):
                        nc.tensor.matmul(
                            sz[b][:, 32 + qc : 33 + qc],
                            xn[b][:, j, qc * 128 : (qc + 1) * 128],
                            esc[b][:, j : j + 1],
                            start=(j == 0),
                            stop=(j == NJ - 1),
                            skip_group_check=True,
                        )
                rse = small_pool.tile([128, 1], F32, tag="rse", name="rse")
                nc.vector.reciprocal(out=rse, in_=sesb[b])
                nc.vector.tensor_scalar_mul(
                    out=zout_s[:, b, :], in0=sz[b][:, 32:36], scalar1=rse
                )

            # ---- software-pipelined batch loop ----------------------------
            xn[0] = xn_pool.tile([128, NJ, D], F8, tag="xn", name="xn")
            nc.sync.dma_start(
                out=xn[0], in_=xn_d[0].rearrange("(j p) q -> p j q", p=128)
            )
            dma_in(1)
            for b in range(PB):
                if b + 2 < PB:
                    dma_in(b + 2)
                # half 0
                for oc in range(KC):
                    emit_g_tanh(b, 0, oc)
                    if oc == 1 and b > 0:
                        emit_scores(b - 1, 1)
                        emit_exp(b - 1)
                    if oc == 3 and b > 0:
                        emit_z_tail(b - 1)
                # half 1
                for oc in range(KC):
                    emit_g_tanh(b, 1, oc)
                    if oc == 1:
                        emit_scores(b, 0)
            # drain the pipeline for the last batch
            emit_scores(PB - 1, 1)
            emit_exp(PB - 1)
            emit_z_tail(PB - 1)

            nc.sync.dma_start(out=z_d[:, :], in_=zout_s)

    nc.compile()
    return nc


def _get_program():
    if "nc" not in _CACHE:
        _CACHE["nc"] = _build_program()
    return _CACHE["nc"]


def kernel(**inputs) -> np.ndarray:
    from concourse import bass_utils

    inp = {k: np.asarray(v) for k, v in inputs.items()}
    input_p = inp["input_p"].astype(np.float32)
    input_q = inp["input_q"].astype(np.float32)
    h_tm1 = inp["h_tm1"].astype(np.float32)
    Wp, Wq, Wr = inp["Wp"], inp["Wq"], inp["Wr"]
    bp, bq, br = inp["bp"], inp["bq"], inp["br"]
    w = inp["w"]

    # shared (weight) tensors
    CWDT = FP8 if CW_FP8 else BF16
    wq8 = np.ascontiguousarray(Wq.T).astype(FP8)
    cw = np.zeros((CROWS, D), dtype=CWDT)
    cw[:D] = Wp.T.astype(CWDT)
    cw[D : 2 * D] = Wr.T.astype(CWDT)
    cw[2 * D] = (bp.astype(np.float32) + bq + br).astype(CWDT)
    wcol = np.ascontiguousarray(w.reshape(D, 1)).astype(BF16)

    nc = _get_program()

    in_maps = []
    for c in range(N_CORES):
        s = slice(c * PB, (c + 1) * PB)
        cx = np.zeros((CROWS, PB), dtype=CWDT)
        cx[:D] = input_p[s].T.astype(CWDT)
        cx[D : 2 * D] = h_tm1[s].T.astype(CWDT)
        cx[2 * D] = 1.0
        xn8 = input_q[s].astype(FP8)
        xt8 = np.ascontiguousarray(xn8.transpose(0, 2, 1))
        in_maps.append(
            {
                "xt8": xt8,
                "xn8": xn8,
                "wq8": wq8,
                "cw": cw,
                "cx": cx,
                "wcol": wcol,
            }
        )

    res = bass_utils.run_bass_kernel_spmd(
        nc, in_maps, core_ids=list(range(N_CORES))
    )
    zs = []
    for c in range(N_CORES):
        zt = np.asarray(res.results[c]["z"], dtype=np.float32).reshape(128, PB, KC)
        zs.append(zt.transpose(1, 2, 0).reshape(PB, D))
    z = np.concatenate(zs, axis=0)
    return np.concatenate([input_p, z], axis=1)


# revision 29
# speedup vs baseline: 2.3917x; 1.0075x over previous
"""MatchLSTM attention kernel for 8 Trainium2 NeuronCores.

Reference computation (B=64, T=2048, D=512):
    G   = tanh(input_p@Wp.T + bp + input_q@Wq.T + bq + h_tm1@Wr.T + br)
    a   = softmax(G@w + match_b)            over T
    z   = sum_t a[:,t] * input_q[:,t,:]
    out = concat([input_p, z], -1)

Sharding: data-parallel over batch, 8 batches per core, weights replicated.

Per-core device pipeline:
  - c^T[o,b] = (Wp.T;Wr.T;bias) matmuls against (ip^T;h^T;ones)  [once]
  - G^T[o,tok] tiles via fp8e4 DoubleRow matmuls (K=256 per instr, 0.5
    cyc/row): Wq^T-chunks (stationary) x X^T-chunks (moving), fp32 PSUM
  - tanh on ScalarE with per-partition bias c^T -> bf16 SBUF  [ACT is the
    bottleneck engine: 8192 free-elems/batch at 0.833 ns/elem]
  - scores come out TRANSPOSED for free: lhsT = tanh-tile [o,128 toks],
    rhs = w-chunk [o,1] -> sT[128,1] per token chunk (1-row matmuls,
    PSUM-accumulated over the 4 o-chunks)
  - one exp per batch on sT[128,16] -> fp8 esc, sumexp via accum_out
  - z^T[q,1] per q-chunk: lhsT = X-natural chunk [tok,128 q], rhs =
    esc[:,j] (1-row matmuls accumulated over the 16 token chunks)
  - sum(pesum) via ones-matmul, reciprocal + scale on VectorE, DMA out.
    Softmax max-subtraction skipped: |s| <= sum|w|+1 < 25, exp safe.
  - X is staged twice from host (fp8 transposed for G, fp8 natural for z)
    so no DMA-transpose is needed; scores/exp/z of batch b-1 are emitted
    inside batch b's G phase so ACT never idles.
"""

import sys

if "/opt/trn_rl_repo" not in sys.path:
    sys.path.insert(0, "/opt/trn_rl_repo")

import numpy as np
import ml_dtypes

N_CORES = 8
B, T, D = 64, 2048, 512
PB = B // N_CORES          # batches per core
KC = D // 128              # 4 contraction / o / q chunks of 128
NJ = T // 128              # 16 token chunks of 128
CROWS = 2 * D + 128        # cw/cx rows: Wp.T, Wr.T, bias row + zero pad
NKC = CROWS // 128         # 9 contraction chunks for the c matmuls

BF16 = ml_dtypes.bfloat16
FP8 = ml_dtypes.float8_e4m3

CW_FP8 = True  # c-projection weights in fp8 (faster startup DMA)

_CACHE: dict = {}


def _build_program():
    import concourse.bacc as bacc
    import concourse.tile as tile
    import concourse.mybir as mybir
    from concourse.bass import MemorySpace

    dt = mybir.dt
    F32 = dt.float32
    BF = dt.bfloat16
    F8 = dt.float8e4
    AF = mybir.ActivationFunctionType
    DR = mybir.MatmulPerfMode.DoubleRow

    nc = bacc.Bacc(
        "TRN2", target_bir_lowering=False, debug=False, num_devices=N_CORES
    )

    xt_d = nc.dram_tensor("xt8", [PB, D, T], F8, kind="ExternalInput")     # X^T
    xn_d = nc.dram_tensor("xn8", [PB, T, D], F8, kind="ExternalInput")     # X
    wq_d = nc.dram_tensor("wq8", [D, D], F8, kind="ExternalInput")         # Wq.T [q,o]
    CWD = F8 if CW_FP8 else BF
    cw_d = nc.dram_tensor("cw", [CROWS, D], CWD, kind="ExternalInput")     # [Wp.T;Wr.T;bias;0]
    cx_d = nc.dram_tensor("cx", [CROWS, PB], CWD, kind="ExternalInput")    # [ip.T;h.T;1;0]
    wcol_d = nc.dram_tensor("wcol", [D, 1], BF, kind="ExternalInput")
    z_d = nc.dram_tensor("z", [128, PB * KC], F32, kind="ExternalOutput")  # z^T chunks

    with tile.TileContext(nc) as tc:
        with (
            tc.tile_pool(name="consts", bufs=1) as consts,
            tc.tile_pool(name="xT_p", bufs=3) as xT_pool,
            tc.tile_pool(name="xn_p", bufs=3) as xn_pool,
            tc.tile_pool(name="th_p", bufs=10) as th_pool,
            tc.tile_pool(name="esc_p", bufs=2) as esc_pool,
            tc.tile_pool(name="small_p", bufs=2) as small_pool,
            tc.tile_pool(name="zout_p", bufs=1) as zout_pool,
            tc.tile_pool(name="pG", bufs=3, space=MemorySpace.PSUM) as pG,
            tc.tile_pool(name="pSZ", bufs=2, space=MemorySpace.PSUM) as pSZ,
        ):
            # ---- constants + batch-0 DMAs, interleaved so the critical path
            # to the first tanh (wq8+xT0[t0:512] for G, then cw+cx for the
            # bias overlapping G's compute) clears the serialized DMA device
            # as early as possible ------------------------------------------
            wq_s = consts.tile([128, KC, D], F8, tag="wq", name="wq_s")
            nc.sync.dma_start(out=wq_s, in_=wq_d.rearrange("(c p) o -> p c o", p=128))
            xT0 = xT_pool.tile([128, KC, T], F8, tag="xT", name="xT")
            nc.sync.dma_start(
                out=xT0[:, :, 0:512],
                in_=xt_d[0, :, 0:512].rearrange("(c p) t -> p c t", p=128),
            )
            cw_s = consts.tile([128, NKC, D], F8 if CW_FP8 else BF, tag="cw", name="cw_s")
            nc.sync.dma_start(out=cw_s, in_=cw_d.rearrange("(c p) o -> p c o", p=128))
            cx_s = consts.tile([128, NKC, PB], F8 if CW_FP8 else BF, tag="cx", name="cx_s")
            nc.sync.dma_start(out=cx_s, in_=cx_d.rearrange("(c p) b -> p c b", p=128))
            nc.sync.dma_start(
                out=xT0[:, :, 512:1024],
                in_=xt_d[0, :, 512:1024].rearrange("(c p) t -> p c t", p=128),
            )
            wcol_s = consts.tile([128, KC, 1], BF, tag="wcol", name="wcol_s")
            nc.sync.dma_start(out=wcol_s, in_=wcol_d.rearrange("(c p) o -> p c o", p=128))
            nc.sync.dma_start(
                out=xT0[:, :, 1024:2048],
                in_=xt_d[0, :, 1024:2048].rearrange("(c p) t -> p c t", p=128),
            )
            ones_bf = consts.tile([128, 128], BF, tag="ones", name="ones_bf")
            nc.vector.memset(ones_bf, 1.0)
            # warm the ACT table set (tanh/exp share one set)
            dummy_s = consts.tile([1, 1], F32, tag="dummy", name="dummy_s")
            nc.scalar.activation(
                out=dummy_s, in_=ones_bf[0:1, 0:1], func=AF.Tanh, bias=0.0, scale=1.0
            )

            # ---- c^T[o, b] for all batches (once); per-oc copy so the
            # first tanh's bias only waits on cw chunk 0 --------------------
            c_ps = pSZ.tile([128, KC, PB], F32, tag="sz", name="c_ps")
            cT_s = consts.tile([128, KC, PB], F32, tag="cT", name="cT_s")
            for oc in range(KC):
                for k in range(NKC):
                    nc.tensor.matmul(
                        c_ps[:, oc, :],
                        cw_s[:, k, oc * 128 : (oc + 1) * 128],
                        cx_s[:, k, :],
                        start=(k == 0),
                        stop=(k == NKC - 1),
                    )
                nc.vector.tensor_copy(out=cT_s[:, oc, :], in_=c_ps[:, oc, :])

            zout_s = zout_pool.tile([128, PB, KC], F32, tag="zout", name="zout_s")

            # per-batch state carried across the software pipeline
            xT = [None] * PB      # [128, KC, T] fp8 transposed X
            xn = [None] * PB      # [128, NJ, D] fp8 natural X
            th = [[None] * (2 * KC) for _ in range(PB)]  # tanh tiles (h*KC+oc)
            sz = [None] * PB      # [128,37] psum: sT 0:16, colsum 16:32, zT 32:36
            esc = [None] * PB     # [128, NJ] exp scores
            sesb = [None] * PB    # [128, 1] f32 sumexp (all partitions)
            xT[0] = xT0

            def dma_xt(b):
                xT[b] = xT_pool.tile([128, KC, T], F8, tag="xT", name="xT")
                nc.sync.dma_start(
                    out=xT[b], in_=xt_d[b].rearrange("(c p) t -> p c t", p=128)
                )

            def dma_xn(b):
                xn[b] = xn_pool.tile([128, NJ, D], F8, tag="xn", name="xn")
                nc.sync.dma_start(
                    out=xn[b], in_=xn_d[b].rearrange("(j p) q -> p j q", p=128)
                )

            def emit_g_tanh(b, h, oc, split=False):
                g = pG.tile([128, 1024], F32, tag="g", name="g_ps")
                t = th_pool.tile([128, 1024], BF, tag="th", name="th")
                # i-outer so each 512-wide PSUM region's accumulation group
                # (kk 0->1) runs start->stop consecutively
                for i in range(2):
                    for kk in range(KC // 2):
                        t0 = h * 1024 + i * 512
                        nc.tensor.matmul(
                            g[:, i * 512 : (i + 1) * 512],
                            wq_s[:, 2 * kk : 2 * kk + 2, oc * 128 : (oc + 1) * 128],
                            xT[b][:, 2 * kk : 2 * kk + 2, t0 : t0 + 512],
                            start=(kk == 0),
                            stop=(kk == KC // 2 - 1),
                            perf_mode=DR,
                        )
                    if split:
                        nc.scalar.activation(
                            out=t[:, i * 512 : (i + 1) * 512],
                            in_=g[:, i * 512 : (i + 1) * 512], func=AF.Tanh,
                            bias=cT_s[:, oc, b : b + 1], scale=1.0,
                        )
                if not split:
                    nc.scalar.activation(
                        out=t, in_=g, func=AF.Tanh,
                        bias=cT_s[:, oc, b : b + 1], scale=1.0,
                    )
                th[b][h * KC + oc] = t

            def emit_scores(b, h):
                if sz[b] is None:
                    sz[b] = pSZ.tile([128, 37], F32, tag="sz", name="sz_ps")
                for jj in range(8):
                    j = h * 8 + jj
                    for oc in range(KC):
                        nc.tensor.matmul(
                            sz[b][:, j : j + 1],
                            th[b][h * KC + oc][:, jj * 128 : (jj + 1) * 128],
                            wcol_s[:, oc, :],
                            start=(oc == 0),
                            stop=(oc == KC - 1),
                            skip_group_check=True,
                        )

            def emit_exp(b):
                esc[b] = esc_pool.tile([128, NJ], BF, tag="esc", name="esc")
                nc.scalar.activation(
                    out=esc[b], in_=sz[b][:, 0:NJ], func=AF.Exp,
                    bias=0.0, scale=1.0,
                )
                # sumexp: per-chunk partition sums on PE (every out partition
                # gets all 16 chunk sums), then a free-dim reduce on DVE
                nc.tensor.matmul(
                    sz[b][:, 16:32], ones_bf, esc[b], start=True, stop=True,
                    skip_group_check=True,
                )
                sesb[b] = small_pool.tile([128, 1], F32, tag="sesb", name="sesb")
                nc.vector.tensor_reduce(
                    out=sesb[b], in_=sz[b][:, 16:32],
                    axis=mybir.AxisListType.X, op=mybir.AluOpType.add,
                )

            def emit_z_tail(b):
                # qc outer: each PSUM column's accumulation group must run
                # start->stop without another group opening in the same bank
                for qc in range(KC):
                    for j in range(NJ):
                        nc.tensor.matmul(
                            sz[b][:, 32 + qc : 33 + qc],
                            xn[b][:, j, qc * 128 : (qc + 1) * 128],
                            esc[b][:, j : j + 1],
                            start=(j == 0),
                            stop=(j == NJ - 1),
                            skip_group_check=True,
                        )
                rse = small_pool.tile([128, 1], F32, tag="rse", name="rse")
                nc.vector.reciprocal(out=rse, in_=sesb[b])
                nc.vector.tensor_scalar_mul(
                    out=zout_s[:, b, :], in0=sz[b][:, 32:36], scalar1=rse
                )

            # ---- software-pipelined batch loop ----------------------------
            # xT(b+1) is prefetched ahead of xn(b): the next batch's G
            # matmuls gate the ACT stream, while z consumes xn only at the
            # end of a batch
            dma_xt(1)
            dma_xn(0)
            dma_xn(1)
            for b in range(PB):
                if b + 2 < PB:
                    dma_xt(b + 2)
                    dma_xn(b + 2)
                # half 0
                for oc in range(KC):
                    emit_g_tanh(b, 0, oc, split=(b == 0 and oc == 0))
                    if oc == 1 and b > 0:
                        emit_scores(b - 1, 1)
                        emit_exp(b - 1)
                    if oc == 3 and b > 0:
                        emit_z_tail(b - 1)
                # half 1
                for oc in range(KC):
                    emit_g_tanh(b, 1, oc)
                    if oc == 1:
                        emit_scores(b, 0)
            # drain the pipeline for the last batch
            emit_scores(PB - 1, 1)
            emit_exp(PB - 1)
            emit_z_tail(PB - 1)

            nc.sync.dma_start(out=z_d[:, :], in_=zout_s)

    nc.compile()
    return nc


def _get_program():
    if "nc" not in _CACHE:
        _CACHE["nc"] = _build_program()
    return _CACHE["nc"]


def kernel(**inputs) -> np.ndarray:
    from concourse import bass_utils

    inp = {k: np.asarray(v) for k, v in inputs.items()}
    input_p = inp["input_p"].astype(np.float32)
    input_q = inp["input_q"].astype(np.float32)
    h_tm1 = inp["h_tm1"].astype(np.float32)
    Wp, Wq, Wr = inp["Wp"], inp["Wq"], inp["Wr"]
    bp, bq, br = inp["bp"], inp["bq"], inp["br"]
    w = inp["w"]

    # shared (weight) tensors
    CWDT = FP8 if CW_FP8 else BF16
    wq8 = np.ascontiguousarray(Wq.T).astype(FP8)
    cw = np.zeros((CROWS, D), dtype=CWDT)
    cw[:D] = Wp.T.astype(CWDT)
    cw[D : 2 * D] = Wr.T.astype(CWDT)
    cw[2 * D] = (bp.astype(np.float32) + bq + br).astype(CWDT)
    wcol = np.ascontiguousarray(w.reshape(D, 1)).astype(BF16)

    nc = _get_program()

    in_maps = []
    for c in range(N_CORES):
        s = slice(c * PB, (c + 1) * PB)
        cx = np.zeros((CROWS, PB), dtype=CWDT)
        cx[:D] = input_p[s].T.astype(CWDT)
        cx[D : 2 * D] = h_tm1[s].T.astype(CWDT)
        cx[2 * D] = 1.0
        xn8 = input_q[s].astype(FP8)
        xt8 = np.ascontiguousarray(xn8.transpose(0, 2, 1))
        in_maps.append(
            {
                "xt8": xt8,
                "xn8": xn8,
                "wq8": wq8,
                "cw": cw,
                "cx": cx,
                "wcol": wcol,
            }
        )

    res = bass_utils.run_bass_kernel_spmd(
        nc, in_maps, core_ids=list(range(N_CORES))
    )
    zs = []
    for c in range(N_CORES):
        zt = np.asarray(res.results[c]["z"], dtype=np.float32).reshape(128, PB, KC)
        zs.append(zt.transpose(1, 2, 0).reshape(PB, D))
    z = np.concatenate(zs, axis=0)
    return np.concatenate([input_p, z], axis=1)
